# revision 22
# baseline (speedup 1.0000x reference)
"""LIF spike kernel for Trainium2 (Bass/Tile), 8-core data-parallel.

v4.2 = v4 baseline structure (engine-local DVE and GPSIMD recurrence
pipelines, ACT/PE shadow work only consumes — no feedback into chains)
plus chunked t-major input streaming:
  - Host pre-permutes each tile's block to [T, sz] so input arrives as
    contiguous 2-step chunks ([P, 2*sz] f32 per DMA). First compute
    starts ~1us in; the DMA stream stays saturated; the drain follows
    the last chunk closely.
All numerics identical to v4 (bit-exact vs the fp32 reference).
"""

import numpy as np

TAU = 0.2
VTH = 0.3

B, C, H, W, T = 32, 128, 32, 32, 8
NCORES = 8
P = 128
SHARD_B = B // NCORES                  # 4 batches per core
VALS = SHARD_B * C * H * W * T         # 4_194_304 values per core
COLS = VALS // P                       # 32768 per partition row
SITES = COLS // T                      # 4096 sites per partition row
Q = 2                                  # t-steps per input DMA chunk

DVE_SLOTS = ((128, 512, 512), (448, 512), (512, 256, 256))
GPS_SLOTS = ((448, 512),)
IO_BUFS = 6                            # chunk ring depth per slot
TMP_BUFS = 3
O_BUFS = 2
COPY_ENGINE = "act"

_cached = None


def _slot_order(dve_slots, gps_slots):
    specs = [("dve", list(s)) for s in dve_slots]
    gspecs = [("gps", list(s)) for s in gps_slots]
    return specs[:1] + gspecs + specs[1:]


def _tiles(dve_slots=DVE_SLOTS, gps_slots=GPS_SLOTS):
    """Global tile list [(off, sz)] with offsets matching the builder:
    dve tiles pack [0, gps_off), gps tiles pack [gps_off, SITES)."""
    gps_sites = sum(sum(s) for s in gps_slots)
    gps_off = SITES - gps_sites
    out = []
    doff, goff = 0, gps_off
    for eng, sizes in _slot_order(dve_slots, gps_slots):
        for sz in sizes:
            if eng == "dve":
                out.append((doff, sz))
                doff += sz
            else:
                out.append((goff, sz))
                goff += sz
    assert doff == gps_off and goff == SITES
    return out


def _make_w():
    import ml_dtypes
    w = np.zeros((P, T * P), dtype=np.float32)
    idx = np.arange(P)
    for t in range(T):
        w[idx, t * P + idx] = np.float32(2.0 ** t)
    return w.astype(ml_dtypes.bfloat16)


def _build_nc(dve_slots=DVE_SLOTS, gps_slots=GPS_SLOTS, io_bufs=IO_BUFS,
              tmp_bufs=TMP_BUFS, o_bufs=O_BUFS, copy_engine=COPY_ENGINE,
              psum_bufs=2, prime_order=(1, 0, 3, 2),
              vt_gps=1965.0, vt_dve=595.0, q_gps=None, io_g_bufs=3):
    import concourse.bass as bass
    import concourse.bacc as bacc
    import concourse.tile as tile
    from concourse import mybir

    f32 = mybir.dt.float32
    bf16 = mybir.dt.bfloat16
    u8 = mybir.dt.uint8
    i8 = mybir.dt.int8
    Alu = mybir.AluOpType
    Act = mybir.ActivationFunctionType

    gps_sites = sum(sum(s) for s in gps_slots)
    gps_off = SITES - gps_sites
    QG = q_gps if q_gps else Q

    nc = bacc.Bacc("TRN2", target_bir_lowering=False, debug=False)
    x = nc.dram_tensor("x", [P, COLS], f32, kind="ExternalInput")
    w = nc.dram_tensor("w", [P, T * P], bf16, kind="ExternalInput")
    o_pk = nc.dram_tensor("o_pk", [P, max(gps_off, 1)], u8,
                          kind="ExternalOutput")
    o_gps = nc.dram_tensor("o_gps", [P, max(gps_sites * T, 1)], i8,
                           kind="ExternalOutput")

    order = _slot_order(dve_slots, gps_slots)

    with tile.TileContext(nc) as tc:
        with (
            tc.tile_pool(name="const", bufs=1) as cpool,
            tc.tile_pool(name="io", bufs=io_bufs) as io_pool,
            tc.tile_pool(name="iog", bufs=io_g_bufs) as iog_pool,
            tc.tile_pool(name="out", bufs=2) as out_pool,
            tc.tile_pool(name="tmp", bufs=tmp_bufs) as tmp_pool,
            tc.tile_pool(name="opool", bufs=o_bufs) as o_pool,
            tc.tile_pool(name="psum", bufs=psum_bufs, space="PSUM") as pp,
        ):
            neg_vth = cpool.tile([P, 1], f32, tag="neg_vth")
            nc.vector.memset(neg_vth[:], -VTH)
            wt = cpool.tile([P, T * P], bf16, tag="w")
            if gps_sites == 0:
                zi = cpool.tile([P, 1], i8, tag="zi")
                nc.vector.memset(zi[:], 0)
                nc.sync.dma_start(o_gps[:, 0:1], zi[:])

            doff, goff = 0, gps_off
            st = []
            for eng, sizes in order:
                sq = QG if eng == "gps" else Q
                nch = T // sq
                tl = []
                for sz in sizes:
                    if eng == "dve":
                        tl.append({"meta": (doff, sz),
                                   "ch": [None] * nch})
                        doff += sz
                    else:
                        tl.append({"meta": (goff, sz),
                                   "ch": [None] * nch})
                        goff += sz
                st.append({"eng": eng, "tiles": tl, "next": 0, "j": None,
                           "t": 0, "u": None, "s": None, "pk": None,
                           "og": None, "sz": 0, "off": 0,
                           "cap": max(sizes), "chunks": None,
                           "issued": 0, "sq": sq, "nch": nch})
            assert doff == gps_off and goff == SITES
            K = len(st)

            def issue_chunk(k):
                s = st[k]
                sq = s["sq"]
                ti, ci = divmod(s["issued"], s["nch"])
                if ti >= len(s["tiles"]):
                    return False
                tl = s["tiles"][ti]
                toff, tsz = tl["meta"]
                pool = iog_pool if (s["eng"] == "gps" and q_gps) \
                    else io_pool
                xin = pool.tile([P, s["cap"] * sq], f32, tag=f"xin{k}")
                base = toff * T + ci * sq * tsz
                nc.sync.dma_start(
                    xin[:, : tsz * sq], x[:, base : base + tsz * sq]
                )
                tl["ch"][ci] = xin
                s["issued"] += 1
                return True

            # prime: io_bufs - 1 chunks per slot, round-robin; the pack
            # weights load after the first round (first matmul is ~4us in)
            porder = (list(prime_order) if prime_order
                      and len(prime_order) == K else list(range(K)))
            for r in range(io_bufs - 1):
                for k in porder:
                    if (st[k]["eng"] == "gps" and q_gps
                            and r >= io_g_bufs - 1):
                        continue
                    issue_chunk(k)
                if r == 0:
                    nc.sync.dma_start(wt[:], w[:, :])

            def xslice(s, t):
                ci, r = divmod(t, s["sq"])
                sz = s["sz"]
                return s["chunks"][ci][:, r * sz : (r + 1) * sz]

            def work_left():
                return any(
                    s["j"] is not None or s["next"] < len(s["tiles"])
                    for s in st
                )

            vt = [0.0] * K

            def step_cost(eng, sz, t):
                per = sz / 512.0
                n = 1 if t in (0, T - 1) else 2
                return per * (vt_gps if eng == "gps" else vt_dve) * n

            while work_left():
                cand = [
                    k for k, s in enumerate(st)
                    if s["j"] is not None or s["next"] < len(s["tiles"])
                ]
                if not cand:
                    break
                k = min(cand, key=lambda k: vt[k])
                s = st[k]
                if s["j"] is None:
                    tl = s["tiles"][s["next"]]
                    s["next"] += 1
                    (s["off"], s["sz"]) = tl["meta"]
                    s["chunks"] = tl["ch"]
                    s["j"], s["t"] = True, 0
                t, sz, eng = s["t"], s["sz"], s["eng"]
                if t % s["sq"] == 0:
                    issue_chunk(k)
                vt[k] += step_cost(eng, sz, t)

                if eng == "gps":
                    if t == 0:
                        s["u"] = xslice(s, 0)
                    else:
                        g = tmp_pool.tile([P, s["cap"]], f32, tag=f"g{k}")
                        nc.gpsimd.tensor_scalar(
                            g[:, :sz], s["s"], TAU, None, Alu.mult
                        )
                        u = tmp_pool.tile([P, s["cap"]], f32, tag=f"u{k}")
                        nc.gpsimd.tensor_tensor(
                            u[:, :sz], g[:, :sz], xslice(s, t), Alu.add
                        )
                        s["u"] = u[:, :sz]
                    if t == 0:
                        og = out_pool.tile([P, s["cap"] * T], i8,
                                           tag=f"og{k}")
                        s["og"] = og
                    ogr = s["og"][:, : sz * T].rearrange(
                        "p (e t) -> p e t", t=T
                    )
                    nc.gpsimd.tensor_scalar(
                        ogr[:, :, t], s["u"], VTH, None, Alu.is_le
                    )
                    if t < T - 1:
                        sn = tmp_pool.tile([P, s["cap"]], f32, tag=f"s{k}")
                        nc.gpsimd.tensor_tensor(
                            sn[:, :sz], ogr[:, :, t], s["u"], Alu.mult
                        )
                        s["s"] = sn[:, :sz]
                        s["t"] += 1
                    else:
                        toff = s["off"] - gps_off
                        nc.sync.dma_start(
                            o_gps[:, toff * T : (toff + sz) * T],
                            s["og"][:, : sz * T],
                        )
                        s["j"] = None
                    continue

                # DVE pipeline
                if t == 0:
                    s["u"] = xslice(s, 0)
                else:
                    u = tmp_pool.tile([P, s["cap"]], f32, tag=f"u{k}")
                    nc.vector.scalar_tensor_tensor(
                        u[:, :sz], s["s"], TAU, xslice(s, t),
                        Alu.mult, Alu.add,
                    )
                    s["u"] = u[:, :sz]
                sg = o_pool.tile([P, s["cap"]], bf16, tag=f"o{k}")
                nc.scalar.activation(
                    sg[:, :sz], s["u"], Act.Sign, bias=neg_vth[:], scale=1.0
                )
                if t == 0:
                    pk = pp.tile([P, s["cap"]], f32, tag=f"pk{k}")
                    s["pk"] = pk
                for c0 in range(0, sz, 512):
                    cs = min(512, sz - c0)
                    nc.tensor.matmul(
                        s["pk"][:, c0 : c0 + cs],
                        wt[:, t * P : (t + 1) * P], sg[:, c0 : c0 + cs],
                        start=(t == 0), stop=(t == T - 1),
                    )
                if t < T - 1:
                    sn = tmp_pool.tile([P, s["cap"]], f32, tag=f"s{k}")
                    nc.vector.scalar_tensor_tensor(
                        sn[:, :sz], s["u"], VTH, s["u"], Alu.is_le, Alu.mult
                    )
                    s["s"] = sn[:, :sz]
                    s["t"] += 1
                else:
                    oi = out_pool.tile([P, s["cap"]], u8, tag=f"out{k}")
                    if copy_engine == "act":
                        nc.scalar.activation(
                            oi[:, :sz], s["pk"][:, :sz], Act.Copy,
                            bias=127.5, scale=0.5,
                        )
                    else:
                        nc.vector.tensor_scalar(
                            oi[:, :sz], s["pk"][:, :sz], 0.5, 127.5,
                            Alu.mult, Alu.add,
                        )
                    nc.sync.dma_start(
                        o_pk[:, s["off"] : s["off"] + sz], oi[:, :sz]
                    )
                    s["j"] = None
    nc.finalize()
    return nc


FALLBACK_CFGS = [
    {},
    {"copy_engine": "dve"},
    {"dve_slots": ((256, 512, 512, 256), (256, 512, 512, 256),
                   (512, 512)), "gps_slots": ()},
]


def _pack_input(xs, dve_slots, gps_slots):
    """xs [NCORES, P, SITES, T] -> t-major per-tile flat [NCORES, P, COLS]."""
    out = np.empty((NCORES, P, COLS), dtype=np.float32)
    for off, sz in _tiles(dve_slots, gps_slots):
        blk = xs[:, :, off:off + sz, :]
        out[:, :, off * T:(off + sz) * T] = (
            blk.transpose(0, 1, 3, 2).reshape(NCORES, P, sz * T)
        )
    return out


def kernel(x):
    global _cached
    from concourse.bass_utils import run_bass_kernel_spmd

    xs = np.ascontiguousarray(np.asarray(x, dtype=np.float32))
    assert xs.shape == (B, C, H, W, T)
    xs = xs.reshape(NCORES, P, SITES, T)
    wnp = _make_w()

    last_err = None
    for cfg in FALLBACK_CFGS:
        try:
            if _cached is None:
                _cached = (_build_nc(**cfg), cfg)
            dve_slots = cfg.get("dve_slots", DVE_SLOTS)
            gps_slots = cfg.get("gps_slots", GPS_SLOTS)
            xp = _pack_input(xs, dve_slots, gps_slots)
            in_maps = [{"x": xp[i], "w": wnp} for i in range(NCORES)]
            res = run_bass_kernel_spmd(_cached[0], in_maps,
                                       list(range(NCORES)))
            cfg = _cached[1]
            break
        except Exception as e:
            last_err = e
            _cached = None
    else:
        raise last_err

    gps_slots = cfg.get("gps_slots", GPS_SLOTS)
    gps_sites = sum(sum(s) for s in gps_slots)
    gps_off = SITES - gps_sites

    out = np.empty((NCORES, P, SITES, T), dtype=np.float32)
    pk = np.stack(
        [res.results[i]["o_pk"] for i in range(NCORES)]
    ).astype(np.uint8, copy=False)
    out[:, :, :gps_off] = np.unpackbits(
        pk[:, :, :gps_off, None], axis=-1, bitorder="little"
    )
    if gps_sites:
        og = np.stack([res.results[i]["o_gps"] for i in range(NCORES)])
        out[:, :, gps_off:] = (
            og[:, :, : gps_sites * T].reshape(NCORES, P, gps_sites, T) == 0
        )
    return out.reshape(B, C, H, W, T)


if __name__ == "__main__":
    rng = np.random.default_rng(0)
    x = rng.standard_normal((B, C, H, W, T), dtype=np.float32)
    out = kernel(x)
    print("out", out.shape, out.dtype, "spike rate", out.mean())


# revision 23
# speedup vs baseline: 1.0049x; 1.0049x over previous
"""LIF spike kernel for Trainium2 (Bass/Tile), 8-core data-parallel.

v4.2 = v4 baseline structure (engine-local DVE and GPSIMD recurrence
pipelines, ACT/PE shadow work only consumes — no feedback into chains)
plus chunked t-major input streaming:
  - Host pre-permutes each tile's block to [T, sz] so input arrives as
    contiguous 2-step chunks ([P, 2*sz] f32 per DMA). First compute
    starts ~1us in; the DMA stream stays saturated; the drain follows
    the last chunk closely.
All numerics identical to v4 (bit-exact vs the fp32 reference).
"""

import numpy as np

TAU = 0.2
VTH = 0.3

B, C, H, W, T = 32, 128, 32, 32, 8
NCORES = 8
P = 128
SHARD_B = B // NCORES                  # 4 batches per core
VALS = SHARD_B * C * H * W * T         # 4_194_304 values per core
COLS = VALS // P                       # 32768 per partition row
SITES = COLS // T                      # 4096 sites per partition row
Q = 2                                  # t-steps per input DMA chunk

DVE_SLOTS = ((192, 448, 512), (512, 448), (512, 256, 256))
GPS_SLOTS = ((448, 512),)
IO_BUFS = 6                            # chunk ring depth per slot
TMP_BUFS = 3
O_BUFS = 2
COPY_ENGINE = "act"

_cached = None


def _slot_order(dve_slots, gps_slots):
    specs = [("dve", list(s)) for s in dve_slots]
    gspecs = [("gps", list(s)) for s in gps_slots]
    return specs[:1] + gspecs + specs[1:]


def _tiles(dve_slots=DVE_SLOTS, gps_slots=GPS_SLOTS):
    """Global tile list [(off, sz)] with offsets matching the builder:
    dve tiles pack [0, gps_off), gps tiles pack [gps_off, SITES)."""
    gps_sites = sum(sum(s) for s in gps_slots)
    gps_off = SITES - gps_sites
    out = []
    doff, goff = 0, gps_off
    for eng, sizes in _slot_order(dve_slots, gps_slots):
        for sz in sizes:
            if eng == "dve":
                out.append((doff, sz))
                doff += sz
            else:
                out.append((goff, sz))
                goff += sz
    assert doff == gps_off and goff == SITES
    return out


def _make_w():
    import ml_dtypes
    w = np.zeros((P, T * P), dtype=np.float32)
    idx = np.arange(P)
    for t in range(T):
        w[idx, t * P + idx] = np.float32(2.0 ** t)
    return w.astype(ml_dtypes.bfloat16)


def _build_nc(dve_slots=DVE_SLOTS, gps_slots=GPS_SLOTS, io_bufs=IO_BUFS,
              tmp_bufs=TMP_BUFS, o_bufs=O_BUFS, copy_engine=COPY_ENGINE,
              psum_bufs=2, prime_order=(1, 0, 3, 2),
              vt_gps=1965.0, vt_dve=595.0, q_gps=None, io_g_bufs=3):
    import concourse.bass as bass
    import concourse.bacc as bacc
    import concourse.tile as tile
    from concourse import mybir

    f32 = mybir.dt.float32
    bf16 = mybir.dt.bfloat16
    u8 = mybir.dt.uint8
    i8 = mybir.dt.int8
    Alu = mybir.AluOpType
    Act = mybir.ActivationFunctionType

    gps_sites = sum(sum(s) for s in gps_slots)
    gps_off = SITES - gps_sites
    QG = q_gps if q_gps else Q

    nc = bacc.Bacc("TRN2", target_bir_lowering=False, debug=False)
    x = nc.dram_tensor("x", [P, COLS], f32, kind="ExternalInput")
    w = nc.dram_tensor("w", [P, T * P], bf16, kind="ExternalInput")
    o_pk = nc.dram_tensor("o_pk", [P, max(gps_off, 1)], u8,
                          kind="ExternalOutput")
    o_gps = nc.dram_tensor("o_gps", [P, max(gps_sites * T, 1)], i8,
                           kind="ExternalOutput")

    order = _slot_order(dve_slots, gps_slots)

    with tile.TileContext(nc) as tc:
        with (
            tc.tile_pool(name="const", bufs=1) as cpool,
            tc.tile_pool(name="io", bufs=io_bufs) as io_pool,
            tc.tile_pool(name="iog", bufs=io_g_bufs) as iog_pool,
            tc.tile_pool(name="out", bufs=2) as out_pool,
            tc.tile_pool(name="tmp", bufs=tmp_bufs) as tmp_pool,
            tc.tile_pool(name="opool", bufs=o_bufs) as o_pool,
            tc.tile_pool(name="psum", bufs=psum_bufs, space="PSUM") as pp,
        ):
            neg_vth = cpool.tile([P, 1], f32, tag="neg_vth")
            nc.vector.memset(neg_vth[:], -VTH)
            wt = cpool.tile([P, T * P], bf16, tag="w")
            if gps_sites == 0:
                zi = cpool.tile([P, 1], i8, tag="zi")
                nc.vector.memset(zi[:], 0)
                nc.sync.dma_start(o_gps[:, 0:1], zi[:])

            doff, goff = 0, gps_off
            st = []
            for eng, sizes in order:
                sq = QG if eng == "gps" else Q
                nch = T // sq
                tl = []
                for sz in sizes:
                    if eng == "dve":
                        tl.append({"meta": (doff, sz),
                                   "ch": [None] * nch})
                        doff += sz
                    else:
                        tl.append({"meta": (goff, sz),
                                   "ch": [None] * nch})
                        goff += sz
                st.append({"eng": eng, "tiles": tl, "next": 0, "j": None,
                           "t": 0, "u": None, "s": None, "pk": None,
                           "og": None, "sz": 0, "off": 0,
                           "cap": max(sizes), "chunks": None,
                           "issued": 0, "sq": sq, "nch": nch})
            assert doff == gps_off and goff == SITES
            K = len(st)

            def issue_chunk(k):
                s = st[k]
                sq = s["sq"]
                ti, ci = divmod(s["issued"], s["nch"])
                if ti >= len(s["tiles"]):
                    return False
                tl = s["tiles"][ti]
                toff, tsz = tl["meta"]
                pool = iog_pool if (s["eng"] == "gps" and q_gps) \
                    else io_pool
                xin = pool.tile([P, s["cap"] * sq], f32, tag=f"xin{k}")
                base = toff * T + ci * sq * tsz
                nc.sync.dma_start(
                    xin[:, : tsz * sq], x[:, base : base + tsz * sq]
                )
                tl["ch"][ci] = xin
                s["issued"] += 1
                return True

            # prime: io_bufs - 1 chunks per slot, round-robin; the pack
            # weights load after the first round (first matmul is ~4us in)
            porder = (list(prime_order) if prime_order
                      and len(prime_order) == K else list(range(K)))
            for r in range(io_bufs - 1):
                for k in porder:
                    if (st[k]["eng"] == "gps" and q_gps
                            and r >= io_g_bufs - 1):
                        continue
                    issue_chunk(k)
                if r == 0:
                    nc.sync.dma_start(wt[:], w[:, :])

            def xslice(s, t):
                ci, r = divmod(t, s["sq"])
                sz = s["sz"]
                return s["chunks"][ci][:, r * sz : (r + 1) * sz]

            def work_left():
                return any(
                    s["j"] is not None or s["next"] < len(s["tiles"])
                    for s in st
                )

            vt = [0.0] * K

            def step_cost(eng, sz, t):
                per = sz / 512.0
                n = 1 if t in (0, T - 1) else 2
                return per * (vt_gps if eng == "gps" else vt_dve) * n

            while work_left():
                cand = [
                    k for k, s in enumerate(st)
                    if s["j"] is not None or s["next"] < len(s["tiles"])
                ]
                if not cand:
                    break
                k = min(cand, key=lambda k: vt[k])
                s = st[k]
                if s["j"] is None:
                    tl = s["tiles"][s["next"]]
                    s["next"] += 1
                    (s["off"], s["sz"]) = tl["meta"]
                    s["chunks"] = tl["ch"]
                    s["j"], s["t"] = True, 0
                t, sz, eng = s["t"], s["sz"], s["eng"]
                if t % s["sq"] == 0:
                    issue_chunk(k)
                vt[k] += step_cost(eng, sz, t)

                if eng == "gps":
                    if t == 0:
                        s["u"] = xslice(s, 0)
                    else:
                        g = tmp_pool.tile([P, s["cap"]], f32, tag=f"g{k}")
                        nc.gpsimd.tensor_scalar(
                            g[:, :sz], s["s"], TAU, None, Alu.mult
                        )
                        u = tmp_pool.tile([P, s["cap"]], f32, tag=f"u{k}")
                        nc.gpsimd.tensor_tensor(
                            u[:, :sz], g[:, :sz], xslice(s, t), Alu.add
                        )
                        s["u"] = u[:, :sz]
                    if t == 0:
                        og = out_pool.tile([P, s["cap"] * T], i8,
                                           tag=f"og{k}")
                        s["og"] = og
                    ogr = s["og"][:, : sz * T].rearrange(
                        "p (e t) -> p e t", t=T
                    )
                    nc.gpsimd.tensor_scalar(
                        ogr[:, :, t], s["u"], VTH, None, Alu.is_le
                    )
                    if t < T - 1:
                        sn = tmp_pool.tile([P, s["cap"]], f32, tag=f"s{k}")
                        nc.gpsimd.tensor_tensor(
                            sn[:, :sz], ogr[:, :, t], s["u"], Alu.mult
                        )
                        s["s"] = sn[:, :sz]
                        s["t"] += 1
                    else:
                        toff = s["off"] - gps_off
                        nc.sync.dma_start(
                            o_gps[:, toff * T : (toff + sz) * T],
                            s["og"][:, : sz * T],
                        )
                        s["j"] = None
                    continue

                # DVE pipeline
                if t == 0:
                    s["u"] = xslice(s, 0)
                else:
                    u = tmp_pool.tile([P, s["cap"]], f32, tag=f"u{k}")
                    nc.vector.scalar_tensor_tensor(
                        u[:, :sz], s["s"], TAU, xslice(s, t),
                        Alu.mult, Alu.add,
                    )
                    s["u"] = u[:, :sz]
                sg = o_pool.tile([P, s["cap"]], bf16, tag=f"o{k}")
                nc.scalar.activation(
                    sg[:, :sz], s["u"], Act.Sign, bias=neg_vth[:], scale=1.0
                )
                if t == 0:
                    pk = pp.tile([P, s["cap"]], f32, tag=f"pk{k}")
                    s["pk"] = pk
                for c0 in range(0, sz, 512):
                    cs = min(512, sz - c0)
                    nc.tensor.matmul(
                        s["pk"][:, c0 : c0 + cs],
                        wt[:, t * P : (t + 1) * P], sg[:, c0 : c0 + cs],
                        start=(t == 0), stop=(t == T - 1),
                    )
                if t < T - 1:
                    sn = tmp_pool.tile([P, s["cap"]], f32, tag=f"s{k}")
                    nc.vector.scalar_tensor_tensor(
                        sn[:, :sz], s["u"], VTH, s["u"], Alu.is_le, Alu.mult
                    )
                    s["s"] = sn[:, :sz]
                    s["t"] += 1
                else:
                    oi = out_pool.tile([P, s["cap"]], u8, tag=f"out{k}")
                    if copy_engine == "act":
                        nc.scalar.activation(
                            oi[:, :sz], s["pk"][:, :sz], Act.Copy,
                            bias=127.5, scale=0.5,
                        )
                    else:
                        nc.vector.tensor_scalar(
                            oi[:, :sz], s["pk"][:, :sz], 0.5, 127.5,
                            Alu.mult, Alu.add,
                        )
                    nc.sync.dma_start(
                        o_pk[:, s["off"] : s["off"] + sz], oi[:, :sz]
                    )
                    s["j"] = None
    nc.finalize()
    return nc


FALLBACK_CFGS = [
    {},
    {"copy_engine": "dve"},
    {"dve_slots": ((256, 512, 512, 256), (256, 512, 512, 256),
                   (512, 512)), "gps_slots": ()},
]


def _pack_input(xs, dve_slots, gps_slots):
    """xs [NCORES, P, SITES, T] -> t-major per-tile flat [NCORES, P, COLS]."""
    out = np.empty((NCORES, P, COLS), dtype=np.float32)
    for off, sz in _tiles(dve_slots, gps_slots):
        blk = xs[:, :, off:off + sz, :]
        out[:, :, off * T:(off + sz) * T] = (
            blk.transpose(0, 1, 3, 2).reshape(NCORES, P, sz * T)
        )
    return out


def kernel(x):
    global _cached
    from concourse.bass_utils import run_bass_kernel_spmd

    xs = np.ascontiguousarray(np.asarray(x, dtype=np.float32))
    assert xs.shape == (B, C, H, W, T)
    xs = xs.reshape(NCORES, P, SITES, T)
    wnp = _make_w()

    last_err = None
    for cfg in FALLBACK_CFGS:
        try:
            if _cached is None:
                _cached = (_build_nc(**cfg), cfg)
            dve_slots = cfg.get("dve_slots", DVE_SLOTS)
            gps_slots = cfg.get("gps_slots", GPS_SLOTS)
            xp = _pack_input(xs, dve_slots, gps_slots)
            in_maps = [{"x": xp[i], "w": wnp} for i in range(NCORES)]
            res = run_bass_kernel_spmd(_cached[0], in_maps,
                                       list(range(NCORES)))
            cfg = _cached[1]
            break
        except Exception as e:
            last_err = e
            _cached = None
    else:
        raise last_err

    gps_slots = cfg.get("gps_slots", GPS_SLOTS)
    gps_sites = sum(sum(s) for s in gps_slots)
    gps_off = SITES - gps_sites

    out = np.empty((NCORES, P, SITES, T), dtype=np.float32)
    pk = np.stack(
        [res.results[i]["o_pk"] for i in range(NCORES)]
    ).astype(np.uint8, copy=False)
    out[:, :, :gps_off] = np.unpackbits(
        pk[:, :, :gps_off, None], axis=-1, bitorder="little"
    )
    if gps_sites:
        og = np.stack([res.results[i]["o_gps"] for i in range(NCORES)])
        out[:, :, gps_off:] = (
            og[:, :, : gps_sites * T].reshape(NCORES, P, gps_sites, T) == 0
        )
    return out.reshape(B, C, H, W, T)


if __name__ == "__main__":
    rng = np.random.default_rng(0)
    x = rng.standard_normal((B, C, H, W, T), dtype=np.float32)
    out = kernel(x)
    print("out", out.shape, out.dtype, "spike rate", out.mean())


# revision 24
# speedup vs baseline: 1.0056x; 1.0007x over previous
"""LIF spike kernel for Trainium2 (Bass/Tile), 8-core data-parallel.

v4.2 = v4 baseline structure (engine-local DVE and GPSIMD recurrence
pipelines, ACT/PE shadow work only consumes — no feedback into chains)
plus chunked t-major input streaming:
  - Host pre-permutes each tile's block to [T, sz] so input arrives as
    contiguous 2-step chunks ([P, 2*sz] f32 per DMA). First compute
    starts ~1us in; the DMA stream stays saturated; the drain follows
    the last chunk closely.
All numerics identical to v4 (bit-exact vs the fp32 reference).
"""

import numpy as np

TAU = 0.2
VTH = 0.3

B, C, H, W, T = 32, 128, 32, 32, 8
NCORES = 8
P = 128
SHARD_B = B // NCORES                  # 4 batches per core
VALS = SHARD_B * C * H * W * T         # 4_194_304 values per core
COLS = VALS // P                       # 32768 per partition row
SITES = COLS // T                      # 4096 sites per partition row
Q = 2                                  # t-steps per input DMA chunk

DVE_SLOTS = ((192, 448, 512), (512, 448), (512, 320, 192))
GPS_SLOTS = ((448, 512),)
IO_BUFS = 6                            # chunk ring depth per slot
TMP_BUFS = 3
O_BUFS = 2
COPY_ENGINE = "act"

_cached = None


def _slot_order(dve_slots, gps_slots):
    specs = [("dve", list(s)) for s in dve_slots]
    gspecs = [("gps", list(s)) for s in gps_slots]
    return specs[:1] + gspecs + specs[1:]


def _tiles(dve_slots=DVE_SLOTS, gps_slots=GPS_SLOTS):
    """Global tile list [(off, sz)] with offsets matching the builder:
    dve tiles pack [0, gps_off), gps tiles pack [gps_off, SITES)."""
    gps_sites = sum(sum(s) for s in gps_slots)
    gps_off = SITES - gps_sites
    out = []
    doff, goff = 0, gps_off
    for eng, sizes in _slot_order(dve_slots, gps_slots):
        for sz in sizes:
            if eng == "dve":
                out.append((doff, sz))
                doff += sz
            else:
                out.append((goff, sz))
                goff += sz
    assert doff == gps_off and goff == SITES
    return out


def _make_w():
    import ml_dtypes
    w = np.zeros((P, T * P), dtype=np.float32)
    idx = np.arange(P)
    for t in range(T):
        w[idx, t * P + idx] = np.float32(2.0 ** t)
    return w.astype(ml_dtypes.bfloat16)


def _build_nc(dve_slots=DVE_SLOTS, gps_slots=GPS_SLOTS, io_bufs=IO_BUFS,
              tmp_bufs=TMP_BUFS, o_bufs=O_BUFS, copy_engine=COPY_ENGINE,
              psum_bufs=2, prime_order=(1, 0, 3, 2),
              vt_gps=1965.0, vt_dve=595.0, q_gps=None, io_g_bufs=3):
    import concourse.bass as bass
    import concourse.bacc as bacc
    import concourse.tile as tile
    from concourse import mybir

    f32 = mybir.dt.float32
    bf16 = mybir.dt.bfloat16
    u8 = mybir.dt.uint8
    i8 = mybir.dt.int8
    Alu = mybir.AluOpType
    Act = mybir.ActivationFunctionType

    gps_sites = sum(sum(s) for s in gps_slots)
    gps_off = SITES - gps_sites
    QG = q_gps if q_gps else Q

    nc = bacc.Bacc("TRN2", target_bir_lowering=False, debug=False)
    x = nc.dram_tensor("x", [P, COLS], f32, kind="ExternalInput")
    w = nc.dram_tensor("w", [P, T * P], bf16, kind="ExternalInput")
    o_pk = nc.dram_tensor("o_pk", [P, max(gps_off, 1)], u8,
                          kind="ExternalOutput")
    o_gps = nc.dram_tensor("o_gps", [P, max(gps_sites * T, 1)], i8,
                           kind="ExternalOutput")

    order = _slot_order(dve_slots, gps_slots)

    with tile.TileContext(nc) as tc:
        with (
            tc.tile_pool(name="const", bufs=1) as cpool,
            tc.tile_pool(name="io", bufs=io_bufs) as io_pool,
            tc.tile_pool(name="iog", bufs=io_g_bufs) as iog_pool,
            tc.tile_pool(name="out", bufs=2) as out_pool,
            tc.tile_pool(name="tmp", bufs=tmp_bufs) as tmp_pool,
            tc.tile_pool(name="opool", bufs=o_bufs) as o_pool,
            tc.tile_pool(name="psum", bufs=psum_bufs, space="PSUM") as pp,
        ):
            neg_vth = cpool.tile([P, 1], f32, tag="neg_vth")
            nc.vector.memset(neg_vth[:], -VTH)
            wt = cpool.tile([P, T * P], bf16, tag="w")
            if gps_sites == 0:
                zi = cpool.tile([P, 1], i8, tag="zi")
                nc.vector.memset(zi[:], 0)
                nc.sync.dma_start(o_gps[:, 0:1], zi[:])

            doff, goff = 0, gps_off
            st = []
            for eng, sizes in order:
                sq = QG if eng == "gps" else Q
                nch = T // sq
                tl = []
                for sz in sizes:
                    if eng == "dve":
                        tl.append({"meta": (doff, sz),
                                   "ch": [None] * nch})
                        doff += sz
                    else:
                        tl.append({"meta": (goff, sz),
                                   "ch": [None] * nch})
                        goff += sz
                st.append({"eng": eng, "tiles": tl, "next": 0, "j": None,
                           "t": 0, "u": None, "s": None, "pk": None,
                           "og": None, "sz": 0, "off": 0,
                           "cap": max(sizes), "chunks": None,
                           "issued": 0, "sq": sq, "nch": nch})
            assert doff == gps_off and goff == SITES
            K = len(st)

            def issue_chunk(k):
                s = st[k]
                sq = s["sq"]
                ti, ci = divmod(s["issued"], s["nch"])
                if ti >= len(s["tiles"]):
                    return False
                tl = s["tiles"][ti]
                toff, tsz = tl["meta"]
                pool = iog_pool if (s["eng"] == "gps" and q_gps) \
                    else io_pool
                xin = pool.tile([P, s["cap"] * sq], f32, tag=f"xin{k}")
                base = toff * T + ci * sq * tsz
                nc.sync.dma_start(
                    xin[:, : tsz * sq], x[:, base : base + tsz * sq]
                )
                tl["ch"][ci] = xin
                s["issued"] += 1
                return True

            # prime: io_bufs - 1 chunks per slot, round-robin; the pack
            # weights load after the first round (first matmul is ~4us in)
            porder = (list(prime_order) if prime_order
                      and len(prime_order) == K else list(range(K)))
            for r in range(io_bufs - 1):
                for k in porder:
                    if (st[k]["eng"] == "gps" and q_gps
                            and r >= io_g_bufs - 1):
                        continue
                    issue_chunk(k)
                if r == 0:
                    nc.sync.dma_start(wt[:], w[:, :])

            def xslice(s, t):
                ci, r = divmod(t, s["sq"])
                sz = s["sz"]
                return s["chunks"][ci][:, r * sz : (r + 1) * sz]

            def work_left():
                return any(
                    s["j"] is not None or s["next"] < len(s["tiles"])
                    for s in st
                )

            vt = [0.0] * K

            def step_cost(eng, sz, t):
                per = sz / 512.0
                n = 1 if t in (0, T - 1) else 2
                return per * (vt_gps if eng == "gps" else vt_dve) * n

            while work_left():
                cand = [
                    k for k, s in enumerate(st)
                    if s["j"] is not None or s["next"] < len(s["tiles"])
                ]
                if not cand:
                    break
                k = min(cand, key=lambda k: vt[k])
                s = st[k]
                if s["j"] is None:
                    tl = s["tiles"][s["next"]]
                    s["next"] += 1
                    (s["off"], s["sz"]) = tl["meta"]
                    s["chunks"] = tl["ch"]
                    s["j"], s["t"] = True, 0
                t, sz, eng = s["t"], s["sz"], s["eng"]
                if t % s["sq"] == 0:
                    issue_chunk(k)
                vt[k] += step_cost(eng, sz, t)

                if eng == "gps":
                    if t == 0:
                        s["u"] = xslice(s, 0)
                    else:
                        g = tmp_pool.tile([P, s["cap"]], f32, tag=f"g{k}")
                        nc.gpsimd.tensor_scalar(
                            g[:, :sz], s["s"], TAU, None, Alu.mult
                        )
                        u = tmp_pool.tile([P, s["cap"]], f32, tag=f"u{k}")
                        nc.gpsimd.tensor_tensor(
                            u[:, :sz], g[:, :sz], xslice(s, t), Alu.add
                        )
                        s["u"] = u[:, :sz]
                    if t == 0:
                        og = out_pool.tile([P, s["cap"] * T], i8,
                                           tag=f"og{k}")
                        s["og"] = og
                    ogr = s["og"][:, : sz * T].rearrange(
                        "p (e t) -> p e t", t=T
                    )
                    nc.gpsimd.tensor_scalar(
                        ogr[:, :, t], s["u"], VTH, None, Alu.is_le
                    )
                    if t < T - 1:
                        sn = tmp_pool.tile([P, s["cap"]], f32, tag=f"s{k}")
                        nc.gpsimd.tensor_tensor(
                            sn[:, :sz], ogr[:, :, t], s["u"], Alu.mult
                        )
                        s["s"] = sn[:, :sz]
                        s["t"] += 1
                    else:
                        toff = s["off"] - gps_off
                        nc.sync.dma_start(
                            o_gps[:, toff * T : (toff + sz) * T],
                            s["og"][:, : sz * T],
                        )
                        s["j"] = None
                    continue

                # DVE pipeline
                if t == 0:
                    s["u"] = xslice(s, 0)
                else:
                    u = tmp_pool.tile([P, s["cap"]], f32, tag=f"u{k}")
                    nc.vector.scalar_tensor_tensor(
                        u[:, :sz], s["s"], TAU, xslice(s, t),
                        Alu.mult, Alu.add,
                    )
                    s["u"] = u[:, :sz]
                sg = o_pool.tile([P, s["cap"]], bf16, tag=f"o{k}")
                nc.scalar.activation(
                    sg[:, :sz], s["u"], Act.Sign, bias=neg_vth[:], scale=1.0
                )
                if t == 0:
                    pk = pp.tile([P, s["cap"]], f32, tag=f"pk{k}")
                    s["pk"] = pk
                for c0 in range(0, sz, 512):
                    cs = min(512, sz - c0)
                    nc.tensor.matmul(
                        s["pk"][:, c0 : c0 + cs],
                        wt[:, t * P : (t + 1) * P], sg[:, c0 : c0 + cs],
                        start=(t == 0), stop=(t == T - 1),
                    )
                if t < T - 1:
                    sn = tmp_pool.tile([P, s["cap"]], f32, tag=f"s{k}")
                    nc.vector.scalar_tensor_tensor(
                        sn[:, :sz], s["u"], VTH, s["u"], Alu.is_le, Alu.mult
                    )
                    s["s"] = sn[:, :sz]
                    s["t"] += 1
                else:
                    oi = out_pool.tile([P, s["cap"]], u8, tag=f"out{k}")
                    if copy_engine == "act":
                        nc.scalar.activation(
                            oi[:, :sz], s["pk"][:, :sz], Act.Copy,
                            bias=127.5, scale=0.5,
                        )
                    else:
                        nc.vector.tensor_scalar(
                            oi[:, :sz], s["pk"][:, :sz], 0.5, 127.5,
                            Alu.mult, Alu.add,
                        )
                    nc.sync.dma_start(
                        o_pk[:, s["off"] : s["off"] + sz], oi[:, :sz]
                    )
                    s["j"] = None
    nc.finalize()
    return nc


FALLBACK_CFGS = [
    {},
    {"copy_engine": "dve"},
    {"dve_slots": ((256, 512, 512, 256), (256, 512, 512, 256),
                   (512, 512)), "gps_slots": ()},
]


def _pack_input(xs, dve_slots, gps_slots):
    """xs [NCORES, P, SITES, T] -> t-major per-tile flat [NCORES, P, COLS]."""
    out = np.empty((NCORES, P, COLS), dtype=np.float32)
    for off, sz in _tiles(dve_slots, gps_slots):
        blk = xs[:, :, off:off + sz, :]
        out[:, :, off * T:(off + sz) * T] = (
            blk.transpose(0, 1, 3, 2).reshape(NCORES, P, sz * T)
        )
    return out


def kernel(x):
    global _cached
    from concourse.bass_utils import run_bass_kernel_spmd

    xs = np.ascontiguousarray(np.asarray(x, dtype=np.float32))
    assert xs.shape == (B, C, H, W, T)
    xs = xs.reshape(NCORES, P, SITES, T)
    wnp = _make_w()

    last_err = None
    for cfg in FALLBACK_CFGS:
        try:
            if _cached is None:
                _cached = (_build_nc(**cfg), cfg)
            dve_slots = cfg.get("dve_slots", DVE_SLOTS)
            gps_slots = cfg.get("gps_slots", GPS_SLOTS)
            xp = _pack_input(xs, dve_slots, gps_slots)
            in_maps = [{"x": xp[i], "w": wnp} for i in range(NCORES)]
            res = run_bass_kernel_spmd(_cached[0], in_maps,
                                       list(range(NCORES)))
            cfg = _cached[1]
            break
        except Exception as e:
            last_err = e
            _cached = None
    else:
        raise last_err

    gps_slots = cfg.get("gps_slots", GPS_SLOTS)
    gps_sites = sum(sum(s) for s in gps_slots)
    gps_off = SITES - gps_sites

    out = np.empty((NCORES, P, SITES, T), dtype=np.float32)
    pk = np.stack(
        [res.results[i]["o_pk"] for i in range(NCORES)]
    ).astype(np.uint8, copy=False)
    out[:, :, :gps_off] = np.unpackbits(
        pk[:, :, :gps_off, None], axis=-1, bitorder="little"
    )
    if gps_sites:
        og = np.stack([res.results[i]["o_gps"] for i in range(NCORES)])
        out[:, :, gps_off:] = (
            og[:, :, : gps_sites * T].reshape(NCORES, P, gps_sites, T) == 0
        )
    return out.reshape(B, C, H, W, T)


if __name__ == "__main__":
    rng = np.random.default_rng(0)
    x = rng.standard_normal((B, C, H, W, T), dtype=np.float32)
    out = kernel(x)
    print("out", out.shape, out.dtype, "spike rate", out.mean())
